# revision 13
# baseline (speedup 1.0000x reference)
"""Modulated conv2d (StyleGAN-2 style, B=16 C=128 HxW=128x128 K=3) on 8 TRN2
NeuronCores, data-parallel over batch (2 samples/core), via 1D Winograd
F(4,3) along W with fp16 matmul operands.

Host-side (pure functions of the inputs):
  s[b,i] = Linear(w)+1 (style), folded into x;  d[b,o] = demod rsqrt, folded
  into the weights.  u[k] = B^T d  (6 components, fp16), g[k,kh] = G-row
  transformed demodulated weight (fp16).  H-padding rows are memset on device.

Device work per 16-row x 32-tile block (16 blocks/core, N=512 per component):
  PE:  18 matmuls (6 comps x 3 kh, K=C_in=128, fp16) -> m0..m5, PSUM banks
       from a rotating 8-bank pool (bank = global_comp_index % 8)
  ACT: c1=m1, c3=m3 (PSUM->SBUF), r4=4r, s2=2s (even blocks), s8=8s
  DVE: p=m2+c1, q=c1-m2, t=m0+p, r=m4+c3, s=c3-m4, w=m5+q  (PSUM TT)
       odd blocks: y1=(s*2)+q (STT)
  GP:  y0=t+r, y2=p+r4, y3=w+s8; even blocks: y1=q+s2      (SBUF TT)
  F(4,3) output identities (nodes 0,+-1,+-2,inf):
       y0=m0+m1+m2+m3+m4, y1=(m1-m2)+2(m3-m4),
       y2=(m1+m2)+4(m3+m4), y3=(m1-m2)+8(m3-m4)+m5.
Four output-column planes (w mod 4) DMA out as fp16; host interleaves.

Engine balance per block (measured): PE ~4.35us, DVE 6x680+STT/2 ~4.5,
GP 3.5x1278 ~4.6, ACT ~3.3.  DVE runs only PSUM-reading ops plus the
odd-block y1 (pure-SBUF DVE ops contend with GPSIMD on the shared SBUF
ports, so y1 alternates DVE/GP by block parity to split the exposure).
All output DMAs issue from the sync engine (q1); ACT's q10 carries the
weights and half the final drain.  22 warmup matmuls bridge the NEFF-boot
to first-input-chunk window so the DVFS clock never de-ramps.  fp16 not
bf16 matmul operands: same PE rate, 8x mantissa -> rel err ~2.4e-3 with
fp32 transform intermediates.
"""

import sys

sys.path.insert(0, "/opt/trn_rl_repo")

import numpy as np

import concourse.bass as bass
from concourse import mybir
from concourse.bass_utils import run_bass_kernel_spmd

B, C, H, W, KS, WD = 16, 128, 128, 128, 3, 512
NCORES = 8
SPC = B // NCORES          # samples per core = 2
HP = H + 2                 # padded rows = 130
TW = W // 4                # output tiles per row (F(4,3)) = 32
NC_COMP = 6                # winograd components
RB = 16                    # output rows per block (N = RB*TW = 512)
NBS = H // RB              # blocks per sample = 8
NB = SPC * NBS             # blocks per core = 16
GRPB = 4                   # blocks per output DMA group
NGRP = NB // GRPB          # 4 output DMA groups
NSLOT = 3                  # output staging slots
NWUP = 22                  # PE warmup matmuls (DVFS ramp)

CHUNKS0 = [0, 17, 33, 49, 81, 128]
CHUNKS1 = [0, 17, 49, 81, 128]

F32 = mybir.dt.float32
F16 = mybir.dt.float16
ADD = mybir.AluOpType.add
SUB = mybir.AluOpType.subtract
MULT = mybir.AluOpType.mult
COPY = mybir.ActivationFunctionType.Copy

COMP_ORDER = [1, 3, 2, 0, 4, 5]   # PE comp order: m1,m3,m2,m0,m4,m5


def dve_ops(gb):
    ops = ["p", "q", "t", "r", "s", "w"]
    if gb % 2:
        ops.append("y1")
    if gb == NB - 2:
        ops.append("y3")        # tail rebalance: DVE absorbs the drain
    if gb == NB - 1:
        ops += ["y0", "y2", "y3"]
    return ops


def gp_ops(gb):
    if gb == NB - 2:
        return ["y0", "y2", "y1"]
    if gb == NB - 1:
        return []
    return ["y0", "y2"] + (["y1"] if gb % 2 == 0 else []) + ["y3"]


def ac_ops(gb):
    return ["c1", "c3", "r4"] + (["s2"] if gb % 2 == 0 else []) + ["s8"]


def _mk_done(sched_fn):
    done, tot = {}, 0
    for gb in range(NB):
        for name in sched_fn(gb):
            tot += 1
            done[(name, gb)] = tot
    return done, tot


DV_DONE, DV_TOT = _mk_done(dve_ops)
GP_DONE, GP_TOT = _mk_done(gp_ops)
AC_DONE, AC_TOT = _mk_done(ac_ops)

FREE_SIG = {0: ("dv", "t"), 1: ("ac", "c1"), 2: ("dv", "q"),
            3: ("ac", "c3"), 4: ("dv", "s"), 5: ("dv", "w")}


def _chunk_for_block(b, bnds):
    need = min(RB * b + RB, H) - 1
    for c in range(len(bnds) - 1):
        if need < bnds[c + 1]:
            return c
    raise AssertionError


def build_program():
    nc = bass.Bass(trn_type="TRN2", target_bir_lowering=False, debug=False)

    u_d = nc.dram_tensor("u", [SPC * C, H, NC_COMP * TW], F16,
                         kind="ExternalInput").ap()
    g_d = nc.dram_tensor("g", [C, SPC * NC_COMP * KS * C], F16,
                         kind="ExternalInput").ap()
    y_d = [nc.dram_tensor(f"y{j}", [SPC * C, H, TW], F16,
                          kind="ExternalOutput").ap() for j in range(4)]

    u_sb = nc.alloc_sbuf_tensor("u_sb", [C, SPC * HP, NC_COMP * TW], F16).ap()
    g_sb = nc.alloc_sbuf_tensor("g_sb", [C, SPC * NC_COMP * KS * C], F16).ap()
    wup = nc.alloc_sbuf_tensor("wup", [C, 640], F16).ap()
    inter = {}
    for name in ("r4", "s2", "s8", "p", "q", "r", "s", "t", "w"):
        inter[name] = nc.alloc_sbuf_tensor(name + "_sb", [C, 2 * 512],
                                           F32).ap()
    # c1/c3 3-deep: WAR reaches gb-3 so ACT evictions never wait on DVE
    for name in ("c1", "c3"):
        inter[name] = nc.alloc_sbuf_tensor(name + "_sb", [C, 3 * 512],
                                           F32).ap()
    ost = nc.alloc_sbuf_tensor("ost", [C, NSLOT * 4 * GRPB * 512], F16).ap()

    pb = [nc.alloc_psum_tensor(f"pb{j}", [C, 512], F32).ap() for j in range(8)]

    s_u = [nc.alloc_semaphore(f"su{i}")
           for i in range(len(CHUNKS0) - 1 + len(CHUNKS1) - 1)]
    s_w0 = nc.alloc_semaphore("s_w0")
    s_w1 = nc.alloc_semaphore("s_w1")
    s_ms = nc.alloc_semaphore("s_ms")
    s_pe = nc.alloc_semaphore("s_pe")
    s_ac = nc.alloc_semaphore("s_ac")
    s_dv = nc.alloc_semaphore("s_dv")
    s_gp = nc.alloc_semaphore("s_gp")
    s_od = [nc.alloc_semaphore(f"sod{i}") for i in range(NSLOT)]

    def gcol(s, k, kh):
        return ((s * NC_COMP + k) * KS + kh) * C

    def isl(name, gb):
        t = inter[name]
        par = gb % (3 if name in ("c1", "c3") else 2)
        return t[:, par * 512:(par + 1) * 512]

    def ost_sl(gb, plane):
        slot = (gb // GRPB) % NSLOT
        j = gb % GRPB
        off = ((slot * 4 + plane) * GRPB + j) * 512
        return ost[:, off:off + 512]

    def grp_geom(grp):
        gs = grp // (NGRP // SPC)
        gr0 = RB * GRPB * (grp % (NGRP // SPC))
        return gs, gr0, grp % NSLOT

    def od_thresh(grp):
        return 64 * (grp // NSLOT)

    with nc.Block() as blk:

        @blk.sync
        def _(eng):
            ci = 0
            for s, bnds in ((0, CHUNKS0), (1, CHUNKS1)):
                for c in range(len(bnds) - 1):
                    r0, r1 = bnds[c], bnds[c + 1]
                    eng.dma_start(
                        out=u_sb[:, s * HP + 1 + r0: s * HP + 1 + r1, :],
                        in_=u_d[s * C:(s + 1) * C, r0:r1, :],
                    ).then_inc(s_u[ci], 16)
                    ci += 1
            for grp in range(NGRP - 1):
                gs, gr0, slot = grp_geom(grp)
                last = GRPB * grp + GRPB - 1
                eng.wait_ge(s_gp, GP_DONE[("y3", last)])
                eng.wait_ge(s_dv, DV_DONE[("y1", last)])
                for plane in range(4):
                    off = ((slot * 4 + plane) * GRPB) * 512
                    eng.dma_start(
                        out=y_d[plane][gs * C:(gs + 1) * C,
                                       gr0:gr0 + RB * GRPB, :],
                        in_=ost[:, off:off + GRPB * 512],
                    ).then_inc(s_od[slot], 16)
            lgrp = NGRP - 1
            ls, lr0, lslot = grp_geom(lgrp)
            gb0 = GRPB * lgrp
            eng.wait_ge(s_gp, GP_DONE[("y3", gb0 + 1)])
            eng.wait_ge(s_dv, DV_DONE[("y1", gb0 + 1)])
            for plane in range(4):
                off = ((lslot * 4 + plane) * GRPB) * 512
                eng.dma_start(
                    out=y_d[plane][ls * C:(ls + 1) * C, lr0:lr0 + 2 * RB, :],
                    in_=ost[:, off:off + 1024],
                ).then_inc(s_od[lslot], 16)
            eng.wait_ge(s_dv, DV_TOT)
            eng.wait_ge(s_gp, GP_TOT)
            for plane in (1, 3):
                off = ((lslot * 4 + plane) * GRPB + 2) * 512
                eng.dma_start(
                    out=y_d[plane][ls * C:(ls + 1) * C,
                                   lr0 + 2 * RB:lr0 + 4 * RB, :],
                    in_=ost[:, off:off + 1024],
                ).then_inc(s_od[lslot], 16)

        @blk.tensor
        def _(eng):
            for i in range(NWUP):
                eng.matmul(out=pb[6 + i % 2], lhsT=wup[:, 0:128],
                           rhs=wup[:, 128:640], start=True, stop=True)
            eng.wait_ge(s_ms, 4)
            eng.wait_ge(s_w0, 16)
            eng.wait_ge(s_u[0], 16)
            for gb in range(NB):
                s, b = gb // NBS, gb % NBS
                bnds = CHUNKS0 if s == 0 else CHUNKS1
                coff = 0 if s == 0 else len(CHUNKS0) - 1
                if gb == NBS:
                    eng.wait_ge(s_w1, 16)
                    eng.wait_ge(s_u[coff], 16)
                c = _chunk_for_block(b, bnds)
                if b > 0 and c != _chunk_for_block(b - 1, bnds):
                    eng.wait_ge(s_u[coff + c], 16)
                for j, comp in enumerate(COMP_ORDER):
                    G = NC_COMP * gb + j
                    bank = G % 8
                    if G >= 8:
                        Gp = G - 8
                        gbp, jp = Gp // NC_COMP, Gp % NC_COMP
                        kind, op = FREE_SIG[COMP_ORDER[jp]]
                        if kind == "ac":
                            eng.wait_ge(s_ac, AC_DONE[(op, gbp)])
                        else:
                            eng.wait_ge(s_dv, DV_DONE[(op, gbp)])
                    for kh in range(KS):
                        inst = eng.matmul(
                            out=pb[bank],
                            lhsT=g_sb[:, gcol(s, comp, kh):
                                      gcol(s, comp, kh) + C],
                            rhs=u_sb[:, s * HP + RB * b + kh:
                                     s * HP + RB * b + kh + RB,
                                     comp * TW:(comp + 1) * TW],
                            start=(kh == 0),
                            stop=(kh == KS - 1),
                        )
                    inst.then_inc(s_pe, 1)

        @blk.scalar
        def _(eng):
            eng.dma_start(out=g_sb[:, 0:NC_COMP * KS * C],
                          in_=g_d[:, 0:NC_COMP * KS * C]).then_inc(s_w0, 16)
            eng.dma_start(out=g_sb[:, NC_COMP * KS * C:],
                          in_=g_d[:, NC_COMP * KS * C:]).then_inc(s_w1, 16)
            for gb in range(NB):
                base = NC_COMP * gb

                def bank(j):
                    return pb[(base + j) % 8]
                # c1 = m1
                if gb >= 3:
                    eng.wait_ge(s_dv, DV_DONE[("q", gb - 3)])
                eng.wait_ge(s_pe, base + 1)
                eng.activation(isl("c1", gb), bank(0), COPY).then_inc(s_ac, 1)
                # c3 = m3 (position 1)
                if gb >= 3:
                    eng.wait_ge(s_dv, DV_DONE[("s", gb - 3)])
                eng.wait_ge(s_pe, base + 2)
                eng.activation(isl("c3", gb), bank(1), COPY).then_inc(s_ac, 1)
                # r4 = 4r
                if gb >= 2:
                    eng.wait_ge(s_gp, GP_DONE[("y2", gb - 2)])
                eng.wait_ge(s_dv, DV_DONE[("r", gb)])
                eng.activation(isl("r4", gb), isl("r", gb), COPY,
                               scale=4.0).then_inc(s_ac, 1)
                # s2 = 2s (even blocks only)
                if gb % 2 == 0:
                    if gb >= 2:
                        eng.wait_ge(s_gp, GP_DONE[("y1", gb - 2)])
                    eng.wait_ge(s_dv, DV_DONE[("s", gb)])
                    eng.activation(isl("s2", gb), isl("s", gb), COPY,
                                   scale=2.0).then_inc(s_ac, 1)
                # s8 = 8s
                if gb >= 2:
                    eng.wait_ge(s_gp, GP_DONE[("y3", gb - 2)])
                eng.wait_ge(s_dv, DV_DONE[("s", gb)])
                eng.activation(isl("s8", gb), isl("s", gb), COPY,
                               scale=8.0).then_inc(s_ac, 1)
            # final drain: planes 0,2 of blocks 14-15 on q10
            lgrp = NGRP - 1
            ls, lr0, lslot = grp_geom(lgrp)
            eng.wait_ge(s_gp, GP_TOT)
            eng.wait_ge(s_dv, DV_TOT)
            for plane in (0, 2):
                off = ((lslot * 4 + plane) * GRPB + 2) * 512
                eng.dma_start(
                    out=y_d[plane][ls * C:(ls + 1) * C,
                                   lr0 + 2 * RB:lr0 + 4 * RB, :],
                    in_=ost[:, off:off + 1024],
                ).then_inc(s_od[lslot], 16)

        @blk.vector
        def _(eng):
            for gb in range(NB):
                base = NC_COMP * gb
                grp, jj = gb // GRPB, gb % GRPB
                slot = grp % NSLOT

                def bank(j):
                    return pb[(base + j) % 8]
                # p = m2 + c1 ; q = c1 - m2   (m2 at position 2)
                if gb >= 2:
                    eng.wait_ge(s_gp, GP_DONE[("y2", gb - 2)])   # WAR p
                eng.wait_ge(s_ac, AC_DONE[("c1", gb)])
                eng.wait_ge(s_pe, base + 3)
                eng.tensor_tensor(isl("p", gb), bank(2), isl("c1", gb),
                                  ADD).then_inc(s_dv, 1)
                if gb >= 2 and gb % 2 == 0:
                    eng.wait_ge(s_gp, GP_DONE[("y1", gb - 2)])   # WAR q
                eng.tensor_tensor(isl("q", gb), isl("c1", gb), bank(2),
                                  SUB).then_inc(s_dv, 1)
                # t = m0 + p   (m0 at position 3)
                eng.wait_ge(s_pe, base + 4)
                eng.tensor_tensor(isl("t", gb), bank(3), isl("p", gb),
                                  ADD).then_inc(s_dv, 1)
                # r = m4 + c3 ; s = c3 - m4
                if gb >= 2:
                    eng.wait_ge(s_gp, GP_DONE[("y0", gb - 2)])   # WAR r
                eng.wait_ge(s_ac, AC_DONE[("c3", gb)])
                eng.wait_ge(s_pe, base + 5)
                eng.tensor_tensor(isl("r", gb), bank(4), isl("c3", gb),
                                  ADD).then_inc(s_dv, 1)
                eng.tensor_tensor(isl("s", gb), isl("c3", gb), bank(4),
                                  SUB).then_inc(s_dv, 1)
                # w = m5 + q
                if gb >= 2:
                    eng.wait_ge(s_gp, GP_DONE[("y3", gb - 2)])   # WAR w
                eng.wait_ge(s_pe, base + 6)
                eng.tensor_tensor(isl("w", gb), bank(5), isl("q", gb),
                                  ADD).then_inc(s_dv, 1)
                # odd blocks: y1 = (s*2) + q
                if gb % 2 == 1:
                    if grp >= NSLOT and jj == 1:
                        eng.wait_ge(s_od[slot], od_thresh(grp))
                    eng.scalar_tensor_tensor(ost_sl(gb, 1), isl("s", gb),
                                             2.0, isl("q", gb), MULT,
                                             ADD).then_inc(s_dv, 1)
                # drain rebalance: DVE finishes the last blocks' planes
                # (GP would lag ~1.5 block-times behind the PE here)
                if gb >= NB - 2:
                    eng.wait_ge(s_ac, AC_DONE[("s8", gb)])
                    if gb == NB - 1:
                        eng.tensor_tensor(ost_sl(gb, 0), isl("t", gb),
                                          isl("r", gb), ADD).then_inc(s_dv, 1)
                        eng.wait_ge(s_ac, AC_DONE[("r4", gb)])
                        eng.tensor_tensor(ost_sl(gb, 2), isl("p", gb),
                                          isl("r4", gb), ADD).then_inc(s_dv,
                                                                       1)
                    eng.tensor_tensor(ost_sl(gb, 3), isl("w", gb),
                                      isl("s8", gb), ADD).then_inc(s_dv, 1)

        @blk.gpsimd
        def _(eng):
            for s in range(SPC):
                for row in (0, HP - 1):
                    eng.memset(u_sb[:, s * HP + row: s * HP + row + 1, :],
                               0.0).then_inc(s_ms, 1)
            for gb in range(NB - 1):
                grp, jj = gb // GRPB, gb % GRPB
                slot = grp % NSLOT
                # y0 = t + r
                if grp >= NSLOT and jj == 0:
                    eng.wait_ge(s_od[slot], od_thresh(grp))
                eng.wait_ge(s_dv, DV_DONE[("r", gb)])
                eng.tensor_tensor(ost_sl(gb, 0), isl("t", gb), isl("r", gb),
                                  ADD).then_inc(s_gp, 1)
                # y2 = p + r4
                eng.wait_ge(s_ac, AC_DONE[("r4", gb)])
                eng.tensor_tensor(ost_sl(gb, 2), isl("p", gb), isl("r4", gb),
                                  ADD).then_inc(s_gp, 1)
                # even blocks: y1 = q + s2
                if gb % 2 == 0:
                    eng.wait_ge(s_ac, AC_DONE[("s2", gb)])
                    eng.tensor_tensor(ost_sl(gb, 1), isl("q", gb),
                                      isl("s2", gb), ADD).then_inc(s_gp, 1)
                # y3 = w + s8 (last two blocks' y3 move to DVE)
                if gb < NB - 2:
                    eng.wait_ge(s_ac, AC_DONE[("s8", gb)])
                    eng.wait_ge(s_dv, DV_DONE[("w", gb)])
                    eng.tensor_tensor(ost_sl(gb, 3), isl("w", gb),
                                      isl("s8", gb), ADD).then_inc(s_gp, 1)

    return nc


# F(4,3) transform matrices, nodes {0, 1, -1, 2, -2, inf}
BT_F43 = np.array([
    [4, 0, -5, 0, 1, 0],
    [0, -4, -4, 1, 1, 0],
    [0, 4, -4, -1, 1, 0],
    [0, -2, -1, 2, 1, 0],
    [0, 2, -1, -2, 1, 0],
    [0, 4, 0, -5, 0, 1]], np.float32)
G_F43 = np.array([
    [1 / 4, 0, 0],
    [-1 / 6, -1 / 6, -1 / 6],
    [-1 / 6, 1 / 6, -1 / 6],
    [1 / 24, 1 / 12, 1 / 6],
    [1 / 24, -1 / 12, 1 / 6],
    [0, 0, 1]], np.float32)


def _host_prep(x, w, weight, mod_w, mod_b):
    f = np.float32
    x = np.asarray(x, f)
    w = np.asarray(w, f)
    weight = np.asarray(weight, f)
    mod_w = np.asarray(mod_w, f)
    mod_b = np.asarray(mod_b, f)

    s_style = (w @ mod_w.T + mod_b) + 1.0
    a_sq = (weight ** 2).sum(axis=(2, 3))
    d = 1.0 / np.sqrt((s_style ** 2) @ a_sq.T + 1e-8)

    in_maps = []
    for core in range(NCORES):
        u_core = np.empty((SPC * C, H, NC_COMP * TW), np.float16)
        g_core = np.empty((C, SPC * NC_COMP * KS * C), np.float16)
        for s in range(SPC):
            b = SPC * core + s
            wd = weight * d[b][:, None, None, None]
            g = np.einsum('cw,oihw->chio', G_F43, wd)
            g_core[:, s * NC_COMP * KS * C:(s + 1) * NC_COMP * KS * C] = (
                g.transpose(2, 0, 1, 3).reshape(C, NC_COMP * KS * C)
                .astype(np.float16))
            xs = x[b] * s_style[b][:, None, None]
            xp = np.zeros((C, H, W + 2), f)
            xp[:, :, 1:W + 1] = xs
            dmat = np.lib.stride_tricks.sliding_window_view(
                xp, 6, axis=2)[:, :, ::4, :]
            u = np.einsum('kd,chtd->chkt', BT_F43, dmat)
            u_core[s * C:(s + 1) * C] = (
                u.reshape(C, H, NC_COMP * TW).astype(np.float16))
        in_maps.append({"u": u_core, "g": g_core})
    return in_maps


def _gather(res):
    y = np.empty((B, C, H, W), np.float32)
    for core in range(NCORES):
        planes = [np.asarray(res.results[core][f"y{j}"]).astype(np.float32)
                  .reshape(SPC, C, H, TW) for j in range(4)]
        yc = np.stack(planes, axis=-1)
        for s in range(SPC):
            y[SPC * core + s] = yc[s].reshape(C, H, W)
    return y


_cached = {}


def kernel(x, w, weight, mod_w, mod_b):
    if "nc" not in _cached:
        _cached["nc"] = build_program()
    nc = _cached["nc"]
    in_maps = _host_prep(x, w, weight, mod_w, mod_b)
    res = run_bass_kernel_spmd(nc, in_maps, list(range(NCORES)))
    return _gather(res)


if __name__ == "__main__":
    from concourse.bass_utils import compile_bass_kernel
    import tempfile

    nc = build_program()
    d = tempfile.mkdtemp()
    neff = compile_bass_kernel(nc, d)
    print("compiled OK:", neff)
